# revision 1
# baseline (speedup 1.0000x reference)
"""DRConv (dynamic region-aware conv) Trainium2 kernel.

Math (per batch b, all on device):
  x_se  = 0.25*sigmoid(routing_w @ mean_hw(x) + routing_b)           # [G*T]
  Z_t   = conv3x3(x, template_t)       for t in 0..T-1               # [O, H, W]
  U     = [x_se.T | 1] contracted with exp(Alpha) over g             # [T+1, P]
  out   = (sum_t Z_t * U_t) / U_T  + bias                            # [O, H, W]
which equals the reference
  out = einsum('boghw,bghw->bohw', einsum('bokg,bkhw->boghw', w, patches),
               softmax(Alpha)) + bias
because w = blend(x_se, templates) commutes through the conv: the blend
weights x_se[g,t] and the softmax probs both act per (g, pixel), so the
G-sum and T-sum exchange with the K-contraction.

Sharding: data-parallel over batch B=8, one batch element per NeuronCore.
Templates/routing weights replicated. No collectives.

Device layout (per core):
  pixels live in a 58x57 plane: one pad row top/bottom, ONE pad column
  (a right-pad column doubles as the left neighbor of the next row's
  x=0 pixel, so 57-wide rows give correct 3x3 zero padding);
  pf = (y+1)*57 + x for image pixel (y, x).
  conv = 9 shifted matmuls accumulating in PSUM:
    Z[px, (t,o)] += x[c, base+px+delta(i,j)].T @ tmpl[c, (t,o)]
  pixel tiles are the stationary operand (128 px per matmul), so the
  per-pixel softmax mixing becomes per-partition scalar_tensor_tensor ops,
  and the final [px, o] -> [o, px] flip is a PE transpose.
"""

import ml_dtypes
import numpy as np

import concourse.bass as bass
import concourse.mybir as mybir
from concourse import bacc, masks
from concourse.tile import TileContext
from concourse.bass_utils import run_bass_kernel_spmd

# problem constants
C = 128          # in channels
O = 128          # out channels
H = W = 56
G = 8            # groups
T = 8            # num weight templates
WP = 57          # padded row width (one shared pad column)
HPAD = 58        # one pad row top and bottom
NPIX = HPAD * WP  # 3306
GUARD = 64       # front guard in the x buffer for negative conv shifts
OFREE = 3328     # 26*128 >= NPIX
PT0 = WP         # first pixel-tile starts at padded row 1
NT = 25          # 25 tiles of 128 px cover pf [57, 3257) > last valid 3247
NCORES = 8

_cache = {}


def _delta(ij):
    i, j = divmod(ij, 3)
    return (i - 1) * WP + (j - 1)


def _build(use_alpha: int):
    f32 = mybir.dt.float32
    bf16 = mybir.dt.bfloat16

    nc = bacc.Bacc("TRN2", target_bir_lowering=False, debug=False,
                   num_devices=NCORES)

    # image ships as bf16 (matmuls are bf16 anyway), split in two row
    # bands so early pixel tiles only wait for the first band
    x0_d = nc.dram_tensor("x0", [C, 31 * W], bf16, kind="ExternalInput")
    x1_d = nc.dram_tensor("x1", [C, 28 * W], bf16, kind="ExternalInput")
    alpha_d = nc.dram_tensor("alpha", [G, H, W], f32, kind="ExternalInput")
    tmpl_d = nc.dram_tensor("tmpl", [9, C, T * O], bf16, kind="ExternalInput")
    rwt_d = nc.dram_tensor("rwt", [C, G * T], f32, kind="ExternalInput")
    rb_d = nc.dram_tensor("rb", [G * T], f32, kind="ExternalInput")
    bias_d = nc.dram_tensor("bias", [O], f32, kind="ExternalInput")
    mask_d = None
    if not use_alpha:
        mask_d = nc.dram_tensor("mask", [H, W], mybir.dt.int32,
                                kind="ExternalInput")
    out_d = nc.dram_tensor("out", [O, OFREE], f32, kind="ExternalOutput")

    with TileContext(nc) as tc:
        with (
            tc.tile_pool(name="big", bufs=1) as big,
            tc.tile_pool(name="consts", bufs=1) as consts,
            tc.tile_pool(name="stage", bufs=3) as stage,
            tc.tile_pool(name="acc", bufs=3) as accp,
            tc.tile_pool(name="upool", bufs=3) as upool,
            tc.tile_pool(name="zps", bufs=3, space="PSUM") as zps,
            tc.tile_pool(name="ups", bufs=1, space="PSUM") as ups,
            tc.tile_pool(name="tps", bufs=1, space="PSUM") as tps,
        ):
            # ---- constants ----
            ident = consts.tile([128, 128], f32)
            masks.make_identity(nc, ident[:])

            # PE warmup: dummy matmuls so HAM un-throttles while the
            # input DMAs stream in (needs only SBUF-resident data)
            warm = tps.tile([128, 128], f32, tag="tp", name="warm")
            for w_i in range(30):
                nc.tensor.matmul(warm[:], lhsT=ident[:], rhs=ident[:])

            bias_rep = consts.tile([128, O], f32)
            nc.sync.dma_start(
                out=bias_rep[:],
                in_=bass.AP(tensor=bias_d, offset=0, ap=[[0, 128], [1, O]]),
            )

            # ---- image band A + routing weights first ----
            xst0 = big.tile([C, 31 * W], bf16)
            nc.sync.dma_start(out=xst0[:], in_=x0_d[:])
            rwt = consts.tile([C, G * T], f32)
            nc.sync.dma_start(out=rwt[:], in_=rwt_d[:])
            rb = consts.tile([G * T, 1], f32)
            nc.sync.dma_start(out=rb[:], in_=rb_d[:])

            # band B DMA too (bf16 bands are small; land them both early)
            XB1 = 29 * WP                  # pf origin of band B buffer
            xst1 = big.tile([C, 28 * W], bf16)
            nc.sync.dma_start(out=xst1[:], in_=x1_d[:])

            # pixel tiles k<=12 read pf [-1, 1779) -> image rows 0..30
            xbf0 = big.tile([C, GUARD + 32 * WP], bf16)
            nc.vector.memset(xbf0[:], 0.0)
            v = xbf0[:, GUARD:GUARD + 32 * WP].rearrange(
                "c (h w) -> c h w", w=WP)
            nc.vector.tensor_copy(
                v[:, 1:32, 0:W], xst0[:].rearrange("c (h w) -> c h w", w=W))

            # ---- templates ----
            tbf = []
            for ij in range(9):
                tb = big.tile([C, T * O], bf16, name=f"tbf{ij}")
                nc.sync.dma_start(out=tb[:], in_=tmpl_d[ij])
                tbf.append(tb)

            # ---- image band B plane: k>=13 read pf [1663, 3315) ----
            xbf1 = big.tile([C, 30 * WP], bf16)
            nc.gpsimd.memset(xbf1[:], 0.0)
            v = xbf1[:, 0:30 * WP].rearrange("c (h w) -> c h w", w=WP)
            nc.gpsimd.tensor_copy(
                v[:, 0:28, 0:W], xst1[:].rearrange("c (h w) -> c h w", w=W))

            # ---- routing: GAP -> fc -> sigmoid (start ASAP) ----
            xsum = consts.tile([C, 1], f32)
            xsum0 = consts.tile([C, 1], f32)
            nc.vector.tensor_reduce(
                out=xsum0[:], in_=xst0[:],
                axis=mybir.AxisListType.X, op=mybir.AluOpType.add)
            nc.vector.tensor_reduce(
                out=xsum[:], in_=xst1[:, 3 * W:],
                axis=mybir.AxisListType.X, op=mybir.AluOpType.add)
            nc.vector.tensor_add(xsum[:], xsum[:], xsum0[:])

            zr = ups.tile([G * T, 1], f32, tag="up")
            nc.tensor.matmul(zr[:], lhsT=rwt[:], rhs=xsum[:])
            # x_se = (2/T)*sigmoid(fc(mean) + rb); mean folded into scale
            xse = consts.tile([G * T, 1], f32)
            nc.scalar.activation(xse[:], zr[:],
                                 mybir.ActivationFunctionType.Sigmoid,
                                 bias=rb[:], scale=1.0 / (H * W))
            xse4 = consts.tile([G * T, 1], bf16)
            nc.vector.tensor_scalar_mul(xse4[:], xse[:], 2.0 / T)

            # lhsT_U [g, T+1]: cols 0..T-1 = x_se[g, t], col T = 1.0
            # (the [64,1] -> [8,8] partition/free reshape is a tiny DMA)
            lhsu = consts.tile([G, T + 1], bf16)
            nc.vector.memset(lhsu[:, T:T + 1], 1.0)
            nc.sync.dma_start(out=lhsu[:, 0:T], in_=xse4[:])

            # ---- routing probability numerators ----
            ea = big.tile([G, OFREE], bf16)
            nc.gpsimd.memset(ea[:], 1.0)
            ea_core = ea[:, 0:NPIX].rearrange("g (h w) -> g h w", w=WP)
            if use_alpha:
                astage = stage.tile([G, H * W], f32, tag="astage")
                nc.sync.dma_start(out=astage[:], in_=alpha_d[:])
                nc.scalar.activation(
                    ea_core[:, 1:57, 0:W],
                    astage[:].rearrange("g (h w) -> g h w", w=W),
                    mybir.ActivationFunctionType.Exp)
            else:
                # hard routing: ea[g, p] = (mask[p] == g)
                mrow = stage.tile([1, H * W], mybir.dt.int32, tag="mrow")
                nc.sync.dma_start(out=mrow[:], in_=mask_d[:])
                mf = stage.tile([1, H * W], f32, tag="mf")
                nc.scalar.copy(mf[:], mrow[:])
                mrep = big.tile([G, H * W], f32)
                for g in range(G):
                    nc.sync.dma_start(out=mrep[g:g + 1, :], in_=mf[:])
                giota = consts.tile([G, 1], f32)
                for g in range(G):
                    nc.vector.memset(giota[g:g + 1, :], float(g))
                nc.vector.tensor_scalar(
                    ea_core[:, 1:57, 0:W],
                    mrep[:].rearrange("g (h w) -> g h w", w=W),
                    giota[:], None, op0=mybir.AluOpType.is_equal)

            # ---- output accumulation plane, 4 window-aligned chunks so
            # stores overlap compute and the tail only waits on the last ----
            OCUT = [0, PT0 + 128 * 7, PT0 + 128 * 13, PT0 + 128 * 19, OFREE]
            outsb = [big.tile([O, OCUT[i + 1] - OCUT[i]], f32,
                              name=f"outsb{i}") for i in range(4)]

            def outsb_slice(lo, n):
                for i in range(4):
                    if lo + n <= OCUT[i + 1]:
                        assert lo >= OCUT[i]
                        return outsb[i][:, lo - OCUT[i]:lo - OCUT[i] + n]
                raise AssertionError(lo)

            # ---- main loop over pixel tiles ----
            for k in range(NT):
                base = PT0 + 128 * k

                up = ups.tile([128, T + 1], f32, tag="up")
                nc.tensor.matmul(up[:], lhsT=ea[:, base:base + 128],
                                 rhs=lhsu[:])
                rcol = upool.tile([128, 1], f32, tag="rcol")
                nc.vector.reciprocal(rcol[:], up[:, T:T + 1])
                usb = upool.tile([128, T], f32, tag="usb")
                nc.vector.tensor_scalar_mul(usb[:], up[:, 0:T], rcol[:])

                zp = [zps.tile([128, 512], f32, tag=f"zp{h}",
                               name=f"zp{h}_{k}")
                      for h in range(2)]
                for ij in range(9):
                    if k <= 12:
                        lo = GUARD + base + _delta(ij)
                        xsl = xbf0[:, lo:lo + 128]
                    else:
                        lo = base - XB1 + _delta(ij)
                        xsl = xbf1[:, lo:lo + 128]
                    for h in range(2):
                        nc.tensor.matmul(
                            zp[h][:],
                            lhsT=xsl,
                            rhs=tbf[ij][:, h * 512:(h + 1) * 512],
                            start=(ij == 0), stop=(ij == 8))

                acc = accp.tile([128, O], f32, tag="acc")
                for t in range(T):
                    h, tq = divmod(t, 4)
                    nc.vector.scalar_tensor_tensor(
                        out=acc[:],
                        in0=zp[h][:, tq * 128:(tq + 1) * 128],
                        scalar=usb[:, t:t + 1],
                        in1=bias_rep[:] if t == 0 else acc[:],
                        op0=mybir.AluOpType.mult,
                        op1=mybir.AluOpType.add)

                tp = tps.tile([128, 128], f32, tag="tp")
                nc.tensor.transpose(tp[:], acc[:], ident[:])
                nc.scalar.copy(outsb_slice(base, 128), tp[:])

            # ---- store padded planes (host strips the padding) ----
            for i in range(4):
                nc.sync.dma_start(out=out_d[:, OCUT[i]:OCUT[i + 1]],
                                  in_=outsb[i][:])

    nc.compile()
    return nc


def _get(use_alpha: int):
    if use_alpha not in _cache:
        _cache[use_alpha] = _build(use_alpha)
    return _cache[use_alpha]


def _in_maps(inp):
    ua = int(np.asarray(inp["use_alpha"]))
    x = np.asarray(inp["inputs"], dtype=np.float32).reshape(
        NCORES, C, H * W).astype(ml_dtypes.bfloat16)
    x0 = np.ascontiguousarray(x[:, :, 0:31 * W])
    x1 = np.ascontiguousarray(x[:, :, 28 * W:])
    Alpha = np.ascontiguousarray(np.asarray(inp["Alpha"], dtype=np.float32))
    # [O*C*3*3, T] -> [(i,j), c, t*O + o]
    tmpl = np.asarray(inp["weight_templates"], dtype=np.float32).reshape(
        O, C, 3, 3, T).transpose(2, 3, 1, 4, 0).reshape(9, C, T * O)
    tmpl = np.ascontiguousarray(tmpl).astype(ml_dtypes.bfloat16)
    rwt = np.ascontiguousarray(
        np.asarray(inp["routing_w"], dtype=np.float32).T)
    rb = np.ascontiguousarray(np.asarray(inp["routing_b"], dtype=np.float32))
    bias = np.ascontiguousarray(np.asarray(inp["bias"], dtype=np.float32))

    in_maps = []
    for b in range(NCORES):
        m = {"x0": x0[b], "x1": x1[b], "alpha": Alpha[b], "tmpl": tmpl,
             "rwt": rwt, "rb": rb, "bias": bias}
        if not ua:
            m["mask"] = np.ascontiguousarray(
                np.asarray(inp["mask"][b], dtype=np.int32))
        in_maps.append(m)
    return in_maps


def kernel(inputs, mask, Alpha, weight_templates, routing_w, routing_b, bias,
           use_alpha):
    ua = int(np.asarray(use_alpha))
    nc = _get(ua)
    in_maps = _in_maps(dict(inputs=inputs, mask=mask, Alpha=Alpha,
                            weight_templates=weight_templates,
                            routing_w=routing_w, routing_b=routing_b,
                            bias=bias, use_alpha=use_alpha))
    res = run_bass_kernel_spmd(nc, in_maps, list(range(NCORES)))
    out = np.stack([res.results[b]["out"] for b in range(NCORES)], axis=0)
    out = out[:, :, :NPIX].reshape(NCORES, O, HPAD, WP)[:, :, 1:57, 0:W]
    return np.ascontiguousarray(out)



# revision 6
# speedup vs baseline: 1.1240x; 1.1240x over previous
"""DRConv (dynamic region-aware conv) Trainium2 kernel.

Math (per batch b, all on device):
  x_se  = 0.25*sigmoid(routing_w @ mean_hw(x) + routing_b)           # [G*T]
  Z_t   = conv3x3(x, template_t)       for t in 0..T-1               # [O, H, W]
  U     = [x_se.T | 1] contracted with exp(Alpha) over g             # [T+1, P]
  out   = (sum_t Z_t * U_t) / U_T  + bias                            # [O, H, W]
which equals the reference because the template blend commutes through
the conv; the G-sum and T-sum exchange with the K-contraction.

Sharding: data-parallel over batch B=8, one batch element per NeuronCore.
Templates/routing weights replicated. No collectives.

Device layout (per core):
  pixels live in a 58x57 plane (one shared pad column), host-prepadded
  so the kernel DMAs the plane directly (no on-device memset/copy);
  pf = (y+1)*57 + x for image pixel (y, x); plane ships with one extra
  guard column in front (offset 1) for the ij=0 shift.
  conv = 9 shifted matmuls accumulating in PSUM per 128-px tile:
    Z[px, (t,o)] += x[c, 1+base+px+delta(i,j)].T @ tmpl[c, (t,o)]
  per-pixel softmax mixing = per-partition scalar_tensor_tensor on DVE,
  final softmax divide folded into one scalar-engine scale per tile;
  output stored [px, O] bf16 and transposed on the host.

Schedule notes (from trace analysis of the previous version):
  - the two HWDGE rings (Sync + Scalar) issue DMAs in parallel;
    template chunks are interleaved across both so conv tile 0 streams
    behind arriving chunks (~0.7us/chunk vs 0.85us of matmul per chunk)
  - routing FC matmul and the 25 batched U-matmuls are placed in the
    PE FIFO after conv tiles 0 and 1 so the PE never idles >3.4us
    (which would re-throttle the HAM clock gate)
  - bf16 warmup matmuls from the first post-preamble slot keep HAM
    warming while input DMAs stream
"""

import ml_dtypes
import numpy as np

import concourse.bass as bass
import concourse.mybir as mybir
from concourse import bacc
from concourse.tile import TileContext
from concourse.bass_utils import run_bass_kernel_spmd

# problem constants
C = 128          # in channels
O = 128          # out channels
H = W = 56
G = 8            # groups
T = 8            # num weight templates
WP = 57          # padded row width (one shared pad column)
HPAD = 58        # one pad row top and bottom
NPIX = HPAD * WP  # 3306
PLANE = 3328     # 1 front guard col + 3306 + tail guard, rounded up
PT0 = 57         # first pixel-tile starts at padded row 1
NT = 25          # 25 tiles of 128 px cover pf [57, 3257) > last valid 3247
XSPL = 1664      # x plane DMA split point (tiles k<=11 only need part A)
NCORES = 8

_cache = {}


def _delta(ij):
    i, j = divmod(ij, 3)
    return (i - 1) * WP + (j - 1)


def _build(use_alpha: int, bias_zero: int):
    f32 = mybir.dt.float32
    bf16 = mybir.dt.bfloat16

    nc = bacc.Bacc("TRN2", target_bir_lowering=False, debug=False,
                   num_devices=NCORES)

    xp_d = nc.dram_tensor("xp", [C, PLANE], bf16, kind="ExternalInput")
    tmpl_d = nc.dram_tensor("tmpl", [9, C, T * O], bf16, kind="ExternalInput")
    rwt_d = nc.dram_tensor("rwt", [C, G * T], f32, kind="ExternalInput")
    rb_d = nc.dram_tensor("rb", [G * T], f32, kind="ExternalInput")
    if use_alpha:
        alpha_d = nc.dram_tensor("alpha", [G, PLANE], f32,
                                 kind="ExternalInput")
    else:
        ea_d = nc.dram_tensor("eain", [G, PLANE], bf16, kind="ExternalInput")
    if not bias_zero:
        bias_d = nc.dram_tensor("bias", [O], f32, kind="ExternalInput")
    out_d = nc.dram_tensor("out", [NT * 128, O], bf16, kind="ExternalOutput")

    with TileContext(nc) as tc:
        with (
            tc.tile_pool(name="big", bufs=1) as big,
            tc.tile_pool(name="accp", bufs=3) as accp,
            tc.tile_pool(name="ps", bufs=1, space="PSUM") as ps,
        ):
            # ---- warmup: bf16 dummies runnable immediately ----
            dummy = big.tile([128, 512], bf16)
            nc.vector.memset(dummy[:], 0.0)
            warm = ps.tile([128, 512], f32, tag="zp", bufs=7, name="warm")
            for _ in range(6):
                nc.tensor.matmul(warm[:], lhsT=dummy[:, 0:128], rhs=dummy[:])

            # ---- DMA issue plan ----
            # sync ring:   xpA, tmpl 0,2,4,6,8, rwt, rb, (lhsu)
            # scalar ring: alpha/ea, xpB, tmpl 1,3,5,7, (out chunks)
            xp = big.tile([C, PLANE], bf16)
            nc.sync.dma_start(out=xp[:, 0:XSPL], in_=xp_d[:, 0:XSPL])

            ea = big.tile([G, PLANE], bf16)
            if use_alpha:
                asb = big.tile([G, PLANE], f32)
                nc.scalar.dma_start(out=asb[:], in_=alpha_d[:])
            else:
                nc.scalar.dma_start(out=ea[:], in_=ea_d[:])
            nc.scalar.dma_start(out=xp[:, XSPL:PLANE],
                                in_=xp_d[:, XSPL:PLANE])

            tbf = []
            for ij in range(9):
                tb = big.tile([C, T * O], bf16, name=f"tbf{ij}")
                tbf.append(tb)
            for ij in (0, 2, 4, 6, 8):
                nc.sync.dma_start(out=tbf[ij][:], in_=tmpl_d[ij])
            for ij in (1, 3, 5, 7):
                nc.scalar.dma_start(out=tbf[ij][:], in_=tmpl_d[ij])

            rwt = big.tile([C, G * T], f32)
            nc.sync.dma_start(out=rwt[:], in_=rwt_d[:])
            rb = big.tile([G * T, 1], f32)
            nc.sync.dma_start(out=rb[:], in_=rb_d[:])
            if not bias_zero:
                bias_rep = big.tile([128, O], f32)
                nc.sync.dma_start(
                    out=bias_rep[:],
                    in_=bass.AP(tensor=bias_d, offset=0,
                                ap=[[0, 128], [1, O]]),
                )

            # ---- routing probability numerators ----
            if use_alpha:
                nc.scalar.activation(ea[:], asb[:],
                                     mybir.ActivationFunctionType.Exp)

            # ---- routing GAP: split reduce so part A starts early ----
            xsA = big.tile([C, 1], f32)
            nc.vector.tensor_reduce(
                out=xsA[:], in_=xp[:, 0:XSPL],
                axis=mybir.AxisListType.X, op=mybir.AluOpType.add)
            xsum = big.tile([C, 1], f32)
            nc.vector.tensor_reduce(
                out=xsum[:], in_=xp[:, XSPL:PLANE],
                axis=mybir.AxisListType.X, op=mybir.AluOpType.add)
            nc.vector.tensor_add(xsum[:], xsum[:], xsA[:])

            # lhsT_U [g, T+1]: cols 0..T-1 = x_se[g, t], col T = 1.0
            lhsu = big.tile([G, T + 1], bf16)
            nc.vector.memset(lhsu[:, T:T + 1], 1.0)

            # U accumulator bank: 25 groups of 9 cols + FC column at 228
            upp = ps.tile([128, 232], f32, tag="up", name="upp")

            # ---- output accumulation ----
            outbuf = big.tile([128, NT * O], bf16)

            def conv_tile(k):
                base = PT0 + 128 * k
                zp = [ps.tile([128, 512], f32, tag="zp", bufs=7,
                              name=f"zp{h}_{k}")
                      for h in range(2)]
                for ij in range(9):
                    lo = 1 + base + _delta(ij)
                    xsl = xp[:, lo:lo + 128]
                    for h in range(2):
                        nc.tensor.matmul(
                            zp[h][:],
                            lhsT=xsl,
                            rhs=tbf[ij][:, h * 512:(h + 1) * 512],
                            start=(ij == 0), stop=(ij == 8))
                return zp

            def mix_tile(k, zp):
                acc = accp.tile([128, O], f32, tag="acc")
                for t in range(T):
                    h, tq = divmod(t, 4)
                    if t == 0:
                        nc.vector.tensor_scalar_mul(
                            acc[:], zp[0][:, 0:128],
                            usb[:, 9 * k:9 * k + 1])
                    else:
                        nc.vector.scalar_tensor_tensor(
                            out=acc[:],
                            in0=zp[h][:, tq * 128:(tq + 1) * 128],
                            scalar=usb[:, t + 9 * k:t + 9 * k + 1],
                            in1=acc[:],
                            op0=mybir.AluOpType.mult,
                            op1=mybir.AluOpType.add)
                # out = acc * (1/U_T); softmax divide folded into the scale
                if bias_zero:
                    nc.scalar.mul(outbuf[:, k * O:(k + 1) * O], acc[:],
                                  rall[:, k:k + 1])
                else:
                    # bias must be added after the U_T divide
                    acc2 = accp.tile([128, O], f32, tag="acc2")
                    nc.scalar.mul(acc2[:], acc[:], rall[:, k:k + 1])
                    nc.vector.tensor_add(outbuf[:, k * O:(k + 1) * O],
                                         acc2[:], bias_rep[:])

            # tiles 0, 1 first; FC after tile 0, U-matmuls after tile 1
            zps = {}
            zps[0] = conv_tile(0)
            # routing FC: zr = rwt.T @ xsum  -> upp[:, 228]
            nc.tensor.matmul(upp[0:G * T, 228:229], lhsT=rwt[:], rhs=xsum[:])
            zps[1] = conv_tile(1)
            # x_se = (2/T)*sigmoid(fc(sum)/HW + rb)
            xse = big.tile([G * T, 1], f32)
            nc.scalar.activation(xse[:], upp[0:G * T, 228:229],
                                 mybir.ActivationFunctionType.Sigmoid,
                                 bias=rb[:], scale=1.0 / (H * W))
            xse4 = big.tile([G * T, 1], bf16)
            nc.vector.tensor_scalar_mul(xse4[:], xse[:], 2.0 / T)
            nc.sync.dma_start(out=lhsu[:, 0:T], in_=xse4[:])

            # batched U matmuls: up[px, (k,t)] for all 25 tiles
            for k in range(NT):
                base = PT0 + 128 * k
                nc.tensor.matmul(upp[:, 9 * k:9 * k + 9],
                                 lhsT=ea[:, base:base + 128], rhs=lhsu[:])

            # 1/U_T for all tiles in one op; copy U to SBUF to free reads
            rall = big.tile([128, NT], f32)
            upv = upp[:, 0:225].rearrange("p (k t) -> p k t", t=9)
            nc.vector.reciprocal(rall[:], upv[:, :, 8])
            usb = big.tile([128, 225], f32)
            nc.scalar.copy(usb[:], upp[:, 0:225])

            mix_tile(0, zps.pop(0))
            zps[2] = conv_tile(2)
            mix_tile(1, zps.pop(1))

            for k in range(3, NT + 1):
                if k <= NT - 1:
                    zps[k] = conv_tile(k)
                mix_tile(k - 1, zps.pop(k - 1))
                # chunked output stores: tiles [6n, 6n+6) per DMA
                done = k  # tiles 0..k-1 mixed
                for n in range(5):
                    if done == min(6 * n + 6, NT):
                        r0 = 6 * n * 128
                        nn = done - 6 * n
                        src = outbuf[:, 6 * n * O:done * O].rearrange(
                            "p (k o) -> p k o", o=O)
                        dst = out_d[r0:r0 + nn * 128, :].rearrange(
                            "(k p) o -> p k o", p=128)
                        nc.scalar.dma_start(out=dst, in_=src)

    nc.compile()
    return nc


def _get(use_alpha: int, bias_zero: int):
    key = (use_alpha, bias_zero)
    if key not in _cache:
        _cache[key] = _build(use_alpha, bias_zero)
    return _cache[key]


def _in_maps(inp):
    ua = int(np.asarray(inp["use_alpha"]))
    bz = int(not np.asarray(inp["bias"]).any())
    x = np.asarray(inp["inputs"], dtype=np.float32).reshape(
        NCORES, C, H, W).astype(ml_dtypes.bfloat16)
    # host-prepadded plane: image row y at pf rows 1..56, cols 0..55,
    # shifted right by 1 guard col
    xp = np.zeros((NCORES, C, PLANE), dtype=ml_dtypes.bfloat16)
    v = xp[:, :, 1:1 + NPIX].reshape(NCORES, C, HPAD, WP)
    v[:, :, 1:57, 0:W] = x
    # [O*C*3*3, T] -> [(i,j), c, t*O + o]
    tmpl = np.asarray(inp["weight_templates"], dtype=np.float32).reshape(
        O, C, 3, 3, T).transpose(2, 3, 1, 4, 0).reshape(9, C, T * O)
    tmpl = np.ascontiguousarray(tmpl).astype(ml_dtypes.bfloat16)
    rwt = np.ascontiguousarray(
        np.asarray(inp["routing_w"], dtype=np.float32).T)
    rb = np.ascontiguousarray(np.asarray(inp["routing_b"], dtype=np.float32))

    if ua:
        ap = np.zeros((NCORES, G, PLANE), dtype=np.float32)
        av = ap[:, :, 1:1 + NPIX].reshape(NCORES, G, HPAD, WP)
        av[:, :, 1:57, 0:W] = np.asarray(inp["Alpha"], dtype=np.float32)
    else:
        # hard routing: one-hot(mask) in plane layout; pads -> group 0
        m = np.asarray(inp["mask"]).astype(np.int64)
        ep = np.zeros((NCORES, G, PLANE), dtype=ml_dtypes.bfloat16)
        ep[:, 0, :] = 1.0
        ev = ep[:, :, 1:1 + NPIX].reshape(NCORES, G, HPAD, WP)
        oh = (m[:, None, :, :] == np.arange(G)[None, :, None, None])
        ev[:, :, 1:57, 0:W] = oh.astype(ml_dtypes.bfloat16)

    in_maps = []
    for b in range(NCORES):
        m = {"xp": xp[b], "tmpl": tmpl, "rwt": rwt, "rb": rb}
        if ua:
            m["alpha"] = ap[b]
        else:
            m["eain"] = ep[b]
        if not bz:
            m["bias"] = np.ascontiguousarray(
                np.asarray(inp["bias"], dtype=np.float32))
        in_maps.append(m)
    return in_maps, ua, bz


_ROWS = (np.arange(H)[:, None] * WP + np.arange(W)[None, :]).ravel()


def kernel(inputs, mask, Alpha, weight_templates, routing_w, routing_b, bias,
           use_alpha):
    in_maps, ua, bz = _in_maps(dict(
        inputs=inputs, mask=mask, Alpha=Alpha,
        weight_templates=weight_templates, routing_w=routing_w,
        routing_b=routing_b, bias=bias, use_alpha=use_alpha))
    nc = _get(ua, bz)
    res = run_bass_kernel_spmd(nc, in_maps, list(range(NCORES)))
    out = np.stack([res.results[b]["out"] for b in range(NCORES)], axis=0)
    # out rows are pf-57; gather valid pixels, transpose to [O, H, W]
    out = np.asarray(out, dtype=np.float32)[:, _ROWS, :]
    out = out.transpose(0, 2, 1).reshape(NCORES, O, H, W)
    return np.ascontiguousarray(out)


# revision 8
# speedup vs baseline: 1.1731x; 1.0437x over previous
"""DRConv (dynamic region-aware conv) Trainium2 kernel.

Math (per batch b, all on device):
  x_se  = 0.25*sigmoid(routing_w @ mean_hw(x) + routing_b)           # [G*T]
  Z_t   = conv3x3(x, template_t)       for t in 0..T-1               # [O, H, W]
  U     = [x_se.T | 1] contracted with exp(Alpha) over g             # [T+1, P]
  out   = sum_t Z_t * (U_t / U_T)  + bias                            # [O, H, W]
which equals the reference because the template blend commutes through
the conv; the G-sum and T-sum exchange with the K-contraction.

Sharding: data-parallel over batch B=8, one batch element per NeuronCore.
Templates/routing weights replicated. No collectives.

Device layout (per core):
  pixels live in a 58x57 plane (one shared pad column), host-prepadded
  so the kernel DMAs the plane directly (no on-device memset/copy);
  pf = (y+1)*57 + x for image pixel (y, x); plane ships with one front
  guard column (image at offset 1) for the ij=0 shift.
  conv = 9 shifted matmuls accumulating in PSUM per 128-px tile:
    Z[px, (t,o)] += x[c, 1+base+px+delta(i,j)].T @ tmpl[c, (t,o)]
  per-pixel softmax mixing = per-partition scalar_tensor_tensor on DVE
  with pre-normalized coefficients U_t/U_T; last stt writes bf16 output
  [px, O] which the host transposes to [O, H, W].

Schedule (from trace analysis):
  - two HWDGE rings (Sync + Scalar) issue DMAs in parallel; template
    chunks alternate rings in conv consumption order
  - tiles 0-2 accumulate ij-outer-interleaved so each arriving template
    chunk feeds 6 matmuls (PE ~85% duty during the input stream)
  - tiles 3-24 run h-major (9 MMs per 512-col half) so the h0 bank
    frees at half-tile and the final tile's mix overlaps its h1 half
  - routing FC + 25 batched U-matmuls sit in the PE FIFO right after
    the interleaved block, so the PE never idles >3.4us (HAM-safe)
"""

import ml_dtypes
import numpy as np

import concourse.bass as bass
import concourse.mybir as mybir
from concourse import bacc
from concourse.tile import TileContext
from concourse.bass_utils import run_bass_kernel_spmd

# problem constants
C = 128          # in channels
O = 128          # out channels
H = W = 56
G = 8            # groups
T = 8            # num weight templates
WP = 57          # padded row width (one shared pad column)
HPAD = 58        # one pad row top and bottom
NPIX = HPAD * WP  # 3306
PLANE = 3328     # 1 front guard col + 3306 + tail guard, rounded up
PT0 = 57         # first pixel-tile starts at padded row 1
NT = 25          # 25 tiles of 128 px cover pf [57, 3257) > last valid 3247
XSPL = 1664      # x plane DMA split point
NFI = 3          # leading tiles accumulated ij-outer during the stream
NCORES = 8

_cache = {}


def _delta(ij):
    i, j = divmod(ij, 3)
    return (i - 1) * WP + (j - 1)


def _build(use_alpha: int, bias_zero: int):
    f32 = mybir.dt.float32
    bf16 = mybir.dt.bfloat16

    nc = bacc.Bacc("TRN2", target_bir_lowering=False, debug=False,
                   num_devices=NCORES)

    xp_d = nc.dram_tensor("xp", [C, PLANE], bf16, kind="ExternalInput")
    tmpl_d = nc.dram_tensor("tmpl", [9, C, T * O], bf16, kind="ExternalInput")
    rwt_d = nc.dram_tensor("rwt", [C, G * T], f32, kind="ExternalInput")
    rb_d = nc.dram_tensor("rb", [G * T], f32, kind="ExternalInput")
    if use_alpha:
        alpha_d = nc.dram_tensor("alpha", [G, PLANE], f32,
                                 kind="ExternalInput")
    else:
        ea_d = nc.dram_tensor("eain", [G, PLANE], bf16, kind="ExternalInput")
    if not bias_zero:
        bias_d = nc.dram_tensor("bias", [O], f32, kind="ExternalInput")
    out_d = nc.dram_tensor("out", [NT * 128, O], bf16, kind="ExternalOutput")

    with TileContext(nc) as tc:
        with (
            tc.tile_pool(name="big", bufs=1) as big,
            tc.tile_pool(name="accp", bufs=3) as accp,
            tc.tile_pool(name="ps", bufs=1, space="PSUM") as ps,
        ):
            # ---- warmup: bf16 dummies runnable immediately ----
            dummy = big.tile([128, 512], bf16)
            nc.vector.memset(dummy[:], 0.0)
            warm = ps.tile([128, 512], f32, tag="zp", bufs=7, name="warm")
            for _ in range(6):
                nc.tensor.matmul(warm[:], lhsT=dummy[:, 0:128], rhs=dummy[:])

            # ---- DMA issue plan ----
            # sync ring:   xpA, tmpl 0,2,4,6,8, rwt, rb, (lhsu, rrep)
            # scalar ring: alpha/ea, tmpl1, xpB, tmpl 3,5,7, (out chunks)
            xp = big.tile([C, PLANE], bf16)
            nc.sync.dma_start(out=xp[:, 0:XSPL], in_=xp_d[:, 0:XSPL])

            ea = big.tile([G, PLANE], bf16)
            if use_alpha:
                asb = big.tile([G, PLANE], f32)
                nc.scalar.dma_start(out=asb[:], in_=alpha_d[:])
            else:
                nc.scalar.dma_start(out=ea[:], in_=ea_d[:])

            tbf = []
            for ij in range(9):
                tb = big.tile([C, T * O], bf16, name=f"tbf{ij}")
                tbf.append(tb)
            nc.sync.dma_start(out=tbf[0][:], in_=tmpl_d[0])
            nc.scalar.dma_start(out=tbf[1][:], in_=tmpl_d[1])
            nc.sync.dma_start(out=tbf[2][:], in_=tmpl_d[2])
            nc.scalar.dma_start(out=xp[:, XSPL:PLANE],
                                in_=xp_d[:, XSPL:PLANE])
            nc.sync.dma_start(out=tbf[4][:], in_=tmpl_d[4])
            nc.scalar.dma_start(out=tbf[3][:], in_=tmpl_d[3])
            nc.sync.dma_start(out=tbf[6][:], in_=tmpl_d[6])
            nc.scalar.dma_start(out=tbf[5][:], in_=tmpl_d[5])
            nc.sync.dma_start(out=tbf[8][:], in_=tmpl_d[8])
            nc.scalar.dma_start(out=tbf[7][:], in_=tmpl_d[7])

            rwt = big.tile([C, G * T], f32)
            nc.sync.dma_start(out=rwt[:], in_=rwt_d[:])
            rb = big.tile([G * T, 1], f32)
            nc.sync.dma_start(out=rb[:], in_=rb_d[:])
            if not bias_zero:
                bias_rep = big.tile([128, O], f32)
                nc.sync.dma_start(
                    out=bias_rep[:],
                    in_=bass.AP(tensor=bias_d, offset=0,
                                ap=[[0, 128], [1, O]]),
                )

            # ---- routing probability numerators ----
            if use_alpha:
                nc.scalar.activation(ea[:], asb[:],
                                     mybir.ActivationFunctionType.Exp)

            # ---- routing GAP: split reduce so part A starts early ----
            xsA = big.tile([C, 1], f32)
            nc.vector.tensor_reduce(
                out=xsA[:], in_=xp[:, 0:XSPL],
                axis=mybir.AxisListType.X, op=mybir.AluOpType.add)
            xsum = big.tile([C, 1], f32)
            nc.vector.tensor_reduce(
                out=xsum[:], in_=xp[:, XSPL:PLANE],
                axis=mybir.AxisListType.X, op=mybir.AluOpType.add)
            nc.vector.tensor_add(xsum[:], xsum[:], xsA[:])

            # lhsT_U [g, T+1]: cols 0..T-1 = x_se[g, t], col T = 1.0
            lhsu = big.tile([G, T + 1], bf16)
            nc.vector.memset(lhsu[:, T:T + 1], 1.0)

            # U accumulator bank (in the zp rotation; freed after the
            # SBUF copy): 25 groups of 9 cols + FC column at 228
            upp = ps.tile([128, 232], f32, tag="zp", bufs=7, name="upp")

            # ---- output staging ----
            outbuf = big.tile([128, NT * O], bf16)

            # ---- leading tiles, ij-outer so each template chunk feeds
            # 2*NFI matmuls while the input stream is still arriving ----
            zps = {k: [ps.tile([128, 512], f32, tag="zp", bufs=7,
                               name=f"zp{h}_{k}") for h in range(2)]
                   for k in range(NFI)}
            for ij in range(9):
                for k in range(NFI):
                    base = PT0 + 128 * k
                    lo = 1 + base + _delta(ij)
                    for h in range(2):
                        nc.tensor.matmul(
                            zps[k][h][:],
                            lhsT=xp[:, lo:lo + 128],
                            rhs=tbf[ij][:, h * 512:(h + 1) * 512],
                            start=(ij == 0), stop=(ij == 8))
                if ij == 5:
                    # routing FC: zr = rwt.T @ xsum  -> upp[:, 228]
                    nc.tensor.matmul(upp[0:G * T, 228:229], lhsT=rwt[:],
                                     rhs=xsum[:])

            # x_se = (2/T)*sigmoid(fc(sum)/HW + rb)
            xse = big.tile([G * T, 1], f32)
            nc.scalar.activation(xse[:], upp[0:G * T, 228:229],
                                 mybir.ActivationFunctionType.Sigmoid,
                                 bias=rb[:], scale=1.0 / (H * W))
            xse4 = big.tile([G * T, 1], bf16)
            nc.vector.tensor_scalar_mul(xse4[:], xse[:], 2.0 / T)
            nc.sync.dma_start(out=lhsu[:, 0:T], in_=xse4[:])

            # batched U matmuls: up[px, (k,t)] for all 25 tiles
            for k in range(NT):
                base = PT0 + 128 * k
                nc.tensor.matmul(upp[:, 9 * k:9 * k + 9],
                                 lhsT=ea[:, base:base + 128], rhs=lhsu[:])

            # normalize once: usb[:, (k,t)] = U_t / U_T per tile
            rall = big.tile([128, NT], f32)
            upv = upp[:, 0:225].rearrange("p (k t) -> p k t", t=9)
            nc.vector.reciprocal(rall[:], upv[:, :, 8])
            usb = big.tile([128, 225], f32)
            nc.vector.tensor_tensor(
                usb[:].rearrange("p (k t) -> p k t", t=9), upv,
                rall[:, :, None].broadcast_to([128, NT, 9]),
                mybir.AluOpType.mult)

            def mix_tile(k, zp):
                acc = accp.tile([128, O], f32, tag="acc")
                for t in range(T):
                    h, tq = divmod(t, 4)
                    dst = acc[:] if t < T - 1 else outbuf[:, k * O:(k + 1) * O]
                    if t == 0:
                        if bias_zero:
                            nc.vector.tensor_scalar_mul(
                                dst, zp[0][:, 0:128], usb[:, 9 * k:9 * k + 1])
                        else:
                            nc.vector.scalar_tensor_tensor(
                                out=dst, in0=zp[0][:, 0:128],
                                scalar=usb[:, 9 * k:9 * k + 1],
                                in1=bias_rep[:],
                                op0=mybir.AluOpType.mult,
                                op1=mybir.AluOpType.add)
                    else:
                        nc.vector.scalar_tensor_tensor(
                            out=dst,
                            in0=zp[h][:, tq * 128:(tq + 1) * 128],
                            scalar=usb[:, t + 9 * k:t + 9 * k + 1],
                            in1=acc[:],
                            op0=mybir.AluOpType.mult,
                            op1=mybir.AluOpType.add)

            def store_chunk(done):
                # chunked output stores: tiles [6n, 6n+6) per DMA
                for n in range(5):
                    if done == min(6 * n + 6, NT):
                        r0 = 6 * n * 128
                        nn = done - 6 * n
                        src = outbuf[:, 6 * n * O:done * O].rearrange(
                            "p (k o) -> p k o", o=O)
                        dst = out_d[r0:r0 + nn * 128, :].rearrange(
                            "(k p) o -> p k o", p=128)
                        nc.scalar.dma_start(out=dst, in_=src)

            for k in range(NFI):
                mix_tile(k, zps.pop(k))
                store_chunk(k + 1)

            # ---- steady state: h-major conv + immediate mix ----
            for k in range(NFI, NT):
                base = PT0 + 128 * k
                zp = [ps.tile([128, 512], f32, tag="zp", bufs=7,
                              name=f"zp{h}_{k}") for h in range(2)]
                for h in range(2):
                    for ij in range(9):
                        lo = 1 + base + _delta(ij)
                        nc.tensor.matmul(
                            zp[h][:],
                            lhsT=xp[:, lo:lo + 128],
                            rhs=tbf[ij][:, h * 512:(h + 1) * 512],
                            start=(ij == 0), stop=(ij == 8))
                mix_tile(k, zp)
                store_chunk(k + 1)

    nc.compile()
    return nc


def _get(use_alpha: int, bias_zero: int):
    key = (use_alpha, bias_zero)
    if key not in _cache:
        _cache[key] = _build(use_alpha, bias_zero)
    return _cache[key]


def _in_maps(inp):
    ua = int(np.asarray(inp["use_alpha"]))
    bz = int(not np.asarray(inp["bias"]).any())
    x = np.asarray(inp["inputs"], dtype=np.float32).reshape(
        NCORES, C, H, W).astype(ml_dtypes.bfloat16)
    # host-prepadded plane: image row y at pf rows 1..56, cols 0..55,
    # shifted right by 1 guard col
    xp = np.zeros((NCORES, C, PLANE), dtype=ml_dtypes.bfloat16)
    v = xp[:, :, 1:1 + NPIX].reshape(NCORES, C, HPAD, WP)
    v[:, :, 1:57, 0:W] = x
    # [O*C*3*3, T] -> [(i,j), c, t*O + o]
    tmpl = np.asarray(inp["weight_templates"], dtype=np.float32).reshape(
        O, C, 3, 3, T).transpose(2, 3, 1, 4, 0).reshape(9, C, T * O)
    tmpl = np.ascontiguousarray(tmpl).astype(ml_dtypes.bfloat16)
    rwt = np.ascontiguousarray(
        np.asarray(inp["routing_w"], dtype=np.float32).T)
    rb = np.ascontiguousarray(np.asarray(inp["routing_b"], dtype=np.float32))

    if ua:
        ap = np.zeros((NCORES, G, PLANE), dtype=np.float32)
        av = ap[:, :, 1:1 + NPIX].reshape(NCORES, G, HPAD, WP)
        av[:, :, 1:57, 0:W] = np.asarray(inp["Alpha"], dtype=np.float32)
    else:
        # hard routing: one-hot(mask) in plane layout; pads -> group 0
        m = np.asarray(inp["mask"]).astype(np.int64)
        ep = np.zeros((NCORES, G, PLANE), dtype=ml_dtypes.bfloat16)
        ep[:, 0, :] = 1.0
        ev = ep[:, :, 1:1 + NPIX].reshape(NCORES, G, HPAD, WP)
        oh = (m[:, None, :, :] == np.arange(G)[None, :, None, None])
        ev[:, :, 1:57, 0:W] = oh.astype(ml_dtypes.bfloat16)

    in_maps = []
    for b in range(NCORES):
        m = {"xp": xp[b], "tmpl": tmpl, "rwt": rwt, "rb": rb}
        if ua:
            m["alpha"] = ap[b]
        else:
            m["eain"] = ep[b]
        if not bz:
            m["bias"] = np.ascontiguousarray(
                np.asarray(inp["bias"], dtype=np.float32))
        in_maps.append(m)
    return in_maps, ua, bz


_ROWS = (np.arange(H)[:, None] * WP + np.arange(W)[None, :]).ravel()


def kernel(inputs, mask, Alpha, weight_templates, routing_w, routing_b, bias,
           use_alpha):
    in_maps, ua, bz = _in_maps(dict(
        inputs=inputs, mask=mask, Alpha=Alpha,
        weight_templates=weight_templates, routing_w=routing_w,
        routing_b=routing_b, bias=bias, use_alpha=use_alpha))
    nc = _get(ua, bz)
    res = run_bass_kernel_spmd(nc, in_maps, list(range(NCORES)))
    out = np.stack([res.results[b]["out"] for b in range(NCORES)], axis=0)
    # out rows are pf-57; gather valid pixels, transpose to [O, H, W]
    out = np.asarray(out, dtype=np.float32)[:, _ROWS, :]
    out = out.transpose(0, 2, 1).reshape(NCORES, O, H, W)
    return np.ascontiguousarray(out)
